# revision 5
# baseline (speedup 1.0000x reference)
"""Masked window self-attention block (Swin-style) — 8-NeuronCore kernel.

Strategy: data-parallel over batch B across 8 cores (4 images/core; windows
are independent per image). Transfers through the axon tunnel dominate wall
time, so the device receives x as fp8e4m3 (25.7MB) and returns only the
residual delta (attn + MLP contribution) as fp8e4m3; the host adds the fp32
shortcut back, which keeps the large identity term at full precision.

Numerics: matmuls in bf16, softmax statistics in fp32. The three softmaxes
(plain / +fg / +bg saliency masks) share one exp() via the factorization
  p + p_fg - p_bg = E * (a + b*g),   a = 1/Z - 1/Zbg, b = 1/Zfg + 1/Zbg
where E = exp(S + rpb + shift_mask), g = per-key fg indicator, Z* = masked
sums of E. Falls back to an exact CPU implementation if no accelerator.
"""

import numpy as np
import ml_dtypes

B, Hh, Ww, C = 32, 56, 56, 256
WIN, SHIFT, HEADS = 7, 3, 8
N = WIN * WIN
nH = Hh // WIN
nW = nH * nH
D = C // HEADS
N_CORES = 8

_BF16 = ml_dtypes.bfloat16
_FP8 = ml_dtypes.float8_e4m3


def _rel_pos_index(w):
    coords = np.stack(np.meshgrid(np.arange(w), np.arange(w), indexing="ij"))
    flat = coords.reshape(2, -1)
    rel = (flat[:, :, None] - flat[:, None, :]).transpose(1, 2, 0).copy()
    rel[..., 0] += w - 1
    rel[..., 1] += w - 1
    rel[..., 0] *= 2 * w - 1
    return rel.sum(-1)


_REL_IDX = _rel_pos_index(WIN)

_CACHE = {}


# fp8e4m3 -> fp32 via 256-entry LUT (much faster than ml_dtypes astype)
_FP8_LUT = np.arange(256, dtype=np.uint8).view(_FP8).astype(np.float32)


def _add_delta(x32, delta8):
    out = np.empty(x32.shape, np.float32)
    np.take(_FP8_LUT, delta8.view(np.uint8), out=out)
    np.add(out, x32, out=out)
    return out


def _get_fn():
    if "fn" in _CACHE:
        return _CACHE["fn"]
    import jax
    import jax.numpy as jnp

    devs = [d for d in jax.devices() if d.platform != "cpu"][:N_CORES]
    if len(devs) < N_CORES:
        raise RuntimeError("need 8 accelerator cores")

    def block(xb, xscale, g_tok, qkv_w, rpb, proj_w, fc1_w, fc2_w, attn_mask,
              n1g, n1b, n2g, n2b, qkv_b, proj_b, fc1_b, fc2_b):
        # xb: (4, 3136, 256) int8 (x quantized by 127/amax); g_tok: (4*nW, N) bool
        f32 = jnp.float32
        x = xb.astype(f32) * xscale
        scale = D ** -0.5
        m = x.mean(-1, keepdims=True)
        v = ((x - m) ** 2).mean(-1, keepdims=True)
        xn = ((x - m) / jnp.sqrt(v + 1e-5) * n1g + n1b).astype(jnp.bfloat16)
        xn = xn.reshape(-1, Hh, Ww, C)
        xs = jnp.roll(xn, (-SHIFT, -SHIFT), axis=(1, 2))
        xw = (xs.reshape(-1, nH, WIN, nH, WIN, C)
              .transpose(0, 1, 3, 2, 4, 5).reshape(-1, N, C))
        B_ = xw.shape[0]
        qkv = (jnp.dot(xw, qkv_w.T) + qkv_b).reshape(B_, N, 3, HEADS, D)
        qkv = qkv.transpose(2, 0, 3, 1, 4).astype(jnp.bfloat16)
        q, k, v_ = qkv[0], qkv[1], qkv[2]
        attn = jnp.einsum("bhnd,bhmd->bhnm", q, k).astype(f32) * scale
        attn = attn + rpb[None]
        attn = (attn.reshape(B_ // nW, nW, HEADS, N, N) + attn_mask[None, :, None]
                ).reshape(B_, HEADS, N, N)
        E = jnp.exp(attn - attn.max(-1, keepdims=True))
        g = g_tok[:, None, None, :].astype(f32)
        Z = E.sum(-1, keepdims=True)
        Zfg = (E * g).sum(-1, keepdims=True)
        Zbg = Z - Zfg
        a = 1.0 / Z - 1.0 / jnp.maximum(Zbg, 1e-30)
        b = 1.0 / jnp.maximum(Zfg, 1e-30) + 1.0 / jnp.maximum(Zbg, 1e-30)
        comb = E * (a + b * g)
        o = jnp.einsum("bhnm,bhmd->bhnd", comb.astype(jnp.bfloat16), v_)
        o = o.astype(jnp.bfloat16).transpose(0, 2, 1, 3).reshape(B_, N, C)
        o = jnp.dot(o, proj_w.T) + proj_b
        xr = (o.reshape(-1, nH, nH, WIN, WIN, C)
              .transpose(0, 1, 3, 2, 4, 5).reshape(-1, Hh, Ww, C))
        xr = jnp.roll(xr, (SHIFT, SHIFT), axis=(1, 2)).reshape(-1, Hh * Ww, C)
        x2 = x + xr.astype(f32)
        m2 = x2.mean(-1, keepdims=True)
        v2 = ((x2 - m2) ** 2).mean(-1, keepdims=True)
        h = ((x2 - m2) / jnp.sqrt(v2 + 1e-5) * n2g + n2b).astype(jnp.bfloat16)
        h1 = jnp.dot(h, fc1_w.T) + fc1_b
        h1 = jax.nn.gelu(h1.astype(f32), approximate=False).astype(jnp.bfloat16)
        h2 = jnp.dot(h1, fc2_w.T) + fc2_b
        return (xr.astype(f32) + h2.astype(f32)).astype(jnp.float8_e4m3)

    fn = jax.pmap(block, devices=devs, in_axes=(0, None, 0) + (None,) * 14)
    _CACHE["fn"] = fn
    return fn


def _block_numpy(x, qkv_w, qkv_b, rpb_table, proj_w, proj_b, norm1_g, norm1_b,
                 norm2_g, norm2_b, fc1_w, fc1_b, fc2_w, fc2_b,
                 attn_mask, sal_fg, sal_bg):
    """Exact float32 reference math (CPU fallback)."""

    def _layer_norm(x, g, b):
        m = x.mean(-1, keepdims=True)
        v = ((x - m) ** 2).mean(-1, keepdims=True)
        return (x - m) / np.sqrt(v + 1e-5) * g + b

    def _softmax(a):
        a = a - a.max(-1, keepdims=True)
        e = np.exp(a)
        return e / e.sum(-1, keepdims=True)

    def _erf(z):
        try:
            from scipy.special import erf as serf
            return serf(z).astype(np.float32)
        except Exception:
            s = np.sign(z)
            a = np.abs(z.astype(np.float64))
            t = 1.0 / (1.0 + 0.3275911 * a)
            y = 1.0 - (((((1.061405429 * t - 1.453152027) * t) + 1.421413741)
                        * t - 0.284496736) * t + 0.254829592) * t * np.exp(-a * a)
            return (s * y).astype(np.float32)

    Bb = x.shape[0]
    scale = np.float32(D ** -0.5)
    shortcut = x
    xn = _layer_norm(x, norm1_g, norm1_b).reshape(Bb, Hh, Ww, C)
    xs = np.roll(xn, (-SHIFT, -SHIFT), axis=(1, 2))
    xw = (xs.reshape(Bb, nH, WIN, nH, WIN, C)
          .transpose(0, 1, 3, 2, 4, 5).reshape(-1, N, C))
    B_ = xw.shape[0]
    qkv = (xw @ qkv_w.T + qkv_b).reshape(B_, N, 3, HEADS, D).transpose(2, 0, 3, 1, 4)
    q, k, v = qkv[0] * scale, qkv[1], qkv[2]
    attn = np.einsum("bhnd,bhmd->bhnm", q, k).astype(np.float32)
    rpb = rpb_table[_REL_IDX.reshape(-1)].reshape(N, N, HEADS).transpose(2, 0, 1)
    attn = attn + rpb[None]
    attn_fg = attn + sal_fg[:, None]
    attn_bg = attn + sal_bg[:, None]

    def add_shift(a):
        a = a.reshape(B_ // nW, nW, HEADS, N, N) + attn_mask[None, :, None]
        return a.reshape(B_, HEADS, N, N)

    p = _softmax(add_shift(attn))
    p_fg = _softmax(add_shift(attn_fg))
    p_bg = _softmax(add_shift(attn_bg))
    o = np.einsum("bhnm,bhmd->bhnd", p + p_fg - p_bg, v).astype(np.float32)
    o = o.transpose(0, 2, 1, 3).reshape(B_, N, C)
    o = o @ proj_w.T + proj_b
    xr = (o.reshape(Bb, nH, nH, WIN, WIN, C)
          .transpose(0, 1, 3, 2, 4, 5).reshape(Bb, Hh, Ww, C))
    xr = np.roll(xr, (SHIFT, SHIFT), axis=(1, 2)).reshape(Bb, Hh * Ww, C)
    x = shortcut + xr
    h = _layer_norm(x, norm2_g, norm2_b)
    h1 = h @ fc1_w.T + fc1_b
    h1 = h1 * 0.5 * (1.0 + _erf(h1 * np.float32(1.0 / np.sqrt(2.0))))
    h = h1 @ fc2_w.T + fc2_b
    return (x + h).astype(np.float32)


def kernel(x, qkv_w, qkv_b, rpb_table, proj_w, proj_b, norm1_g, norm1_b,
           norm2_g, norm2_b, fc1_w, fc1_b, fc2_w, fc2_b,
           attn_mask, sal_fg_attn_mask, sal_bg_attn_mask):
    x32 = np.ascontiguousarray(np.asarray(x, np.float32))
    try:
        fn = _get_fn()
        amax = float(max(-x32.min(), x32.max(), 1e-6))
        q = np.float32(127.0 / amax)
        xb = np.empty(x32.shape, np.int8)
        np.multiply(x32, q, out=_CACHE.setdefault("tmp32", np.empty(x32.shape, np.float32)))
        np.rint(_CACHE["tmp32"], out=_CACHE["tmp32"])
        xb = _CACHE["tmp32"].astype(np.int8)
        xb = xb.reshape(N_CORES, B // N_CORES, Hh * Ww, C)
        g_tok = (np.asarray(sal_fg_attn_mask)[:, 0, :] == 0.0)
        g_tok = np.ascontiguousarray(g_tok).reshape(N_CORES, (B // N_CORES) * nW, N)
        rpb = np.asarray(rpb_table, np.float32)[_REL_IDX.reshape(-1)]
        rpb = np.ascontiguousarray(rpb.reshape(N, N, HEADS).transpose(2, 0, 1))
        delta8 = np.asarray(fn(
            xb, np.float32(1.0 / q), g_tok,
            np.asarray(qkv_w, np.float32).astype(_BF16), rpb,
            np.asarray(proj_w, np.float32).astype(_BF16),
            np.asarray(fc1_w, np.float32).astype(_BF16),
            np.asarray(fc2_w, np.float32).astype(_BF16),
            np.asarray(attn_mask, np.float32),
            np.asarray(norm1_g, np.float32), np.asarray(norm1_b, np.float32),
            np.asarray(norm2_g, np.float32), np.asarray(norm2_b, np.float32),
            np.asarray(qkv_b, np.float32), np.asarray(proj_b, np.float32),
            np.asarray(fc1_b, np.float32), np.asarray(fc2_b, np.float32),
        ))
        return _add_delta(x32, delta8.reshape(B, Hh * Ww, C))
    except Exception:
        pass

    # CPU fallback: exact math, sharded over batch.
    args = [np.asarray(a, np.float32) for a in (
        qkv_w, qkv_b, rpb_table, proj_w, proj_b, norm1_g, norm1_b,
        norm2_g, norm2_b, fc1_w, fc1_b, fc2_w, fc2_b, attn_mask)]
    sal_fg = np.asarray(sal_fg_attn_mask, np.float32)
    sal_bg = np.asarray(sal_bg_attn_mask, np.float32)
    outs = []
    per = B // N_CORES
    for c in range(N_CORES):
        xs = x32[c * per:(c + 1) * per]
        sf = sal_fg[c * per * nW:(c + 1) * per * nW]
        sb = sal_bg[c * per * nW:(c + 1) * per * nW]
        outs.append(_block_numpy(xs, *args[:14], sf, sb))
    return np.concatenate(outs, 0)


# revision 7
# speedup vs baseline: 1.2816x; 1.2816x over previous
"""Masked window self-attention block (Swin-style) — 8-NeuronCore kernel.

Strategy: data-parallel over batch B across 8 cores (4 images/core; windows
are independent per image). Transfers through the axon tunnel dominate wall
time, so the device receives x as fp8e4m3 (25.7MB) and returns only the
residual delta (attn + MLP contribution) as fp8e4m3; the host adds the fp32
shortcut back, which keeps the large identity term at full precision.

Numerics: matmuls in bf16, softmax statistics in fp32. The three softmaxes
(plain / +fg / +bg saliency masks) share one exp() via the factorization
  p + p_fg - p_bg = E * (a + b*g),   a = 1/Z - 1/Zbg, b = 1/Zfg + 1/Zbg
where E = exp(S + rpb + shift_mask), g = per-key fg indicator, Z* = masked
sums of E. Falls back to an exact CPU implementation if no accelerator.
"""

import numpy as np
import ml_dtypes

B, Hh, Ww, C = 32, 56, 56, 256
WIN, SHIFT, HEADS = 7, 3, 8
N = WIN * WIN
nH = Hh // WIN
nW = nH * nH
D = C // HEADS
N_CORES = 8

_BF16 = ml_dtypes.bfloat16
_FP8 = ml_dtypes.float8_e4m3


def _rel_pos_index(w):
    coords = np.stack(np.meshgrid(np.arange(w), np.arange(w), indexing="ij"))
    flat = coords.reshape(2, -1)
    rel = (flat[:, :, None] - flat[:, None, :]).transpose(1, 2, 0).copy()
    rel[..., 0] += w - 1
    rel[..., 1] += w - 1
    rel[..., 0] *= 2 * w - 1
    return rel.sum(-1)


_REL_IDX = _rel_pos_index(WIN)

_CACHE = {}


# fp8e4m3 -> fp32 via 256-entry LUT (much faster than ml_dtypes astype)
_FP8_LUT = np.arange(256, dtype=np.uint8).view(_FP8).astype(np.float32)


def _add_delta(x32, delta8):
    out = np.empty(x32.shape, np.float32)
    np.take(_FP8_LUT, delta8.view(np.uint8), out=out)
    np.add(out, x32, out=out)
    return out


def _get_fn():
    if "fn" in _CACHE:
        return _CACHE["fn"]
    import jax
    import jax.numpy as jnp

    devs = [d for d in jax.devices() if d.platform != "cpu"][:N_CORES]
    if len(devs) < N_CORES:
        raise RuntimeError("need 8 accelerator cores")

    def block(xb, xscale, g_tok, qkv_w, rpb, proj_w, fc1_w, fc2_w, attn_mask,
              n1g, n1b, n2g, n2b, qkv_b, proj_b, fc1_b, fc2_b):
        # xb: (4, 3136, 256) int8 (x quantized by 127/amax); g_tok: (4*nW, N) bool
        f32 = jnp.float32
        x = xb.astype(f32) * xscale
        scale = D ** -0.5
        m = x.mean(-1, keepdims=True)
        v = ((x - m) ** 2).mean(-1, keepdims=True)
        xn = ((x - m) / jnp.sqrt(v + 1e-5) * n1g + n1b).astype(jnp.bfloat16)
        xn = xn.reshape(-1, Hh, Ww, C)
        xs = jnp.roll(xn, (-SHIFT, -SHIFT), axis=(1, 2))
        xw = (xs.reshape(-1, nH, WIN, nH, WIN, C)
              .transpose(0, 1, 3, 2, 4, 5).reshape(-1, N, C))
        B_ = xw.shape[0]
        qkv = (jnp.dot(xw, qkv_w.T) + qkv_b).reshape(B_, N, 3, HEADS, D)
        qkv = qkv.transpose(2, 0, 3, 1, 4).astype(jnp.bfloat16)
        q, k, v_ = qkv[0], qkv[1], qkv[2]
        attn = jnp.einsum("bhnd,bhmd->bhnm", q, k).astype(f32) * scale
        attn = attn + rpb[None]
        attn = (attn.reshape(B_ // nW, nW, HEADS, N, N) + attn_mask[None, :, None]
                ).reshape(B_, HEADS, N, N)
        E = jnp.exp(attn - attn.max(-1, keepdims=True))
        g = g_tok[:, None, None, :].astype(f32)
        Z = E.sum(-1, keepdims=True)
        Zfg = (E * g).sum(-1, keepdims=True)
        Zbg = Z - Zfg
        a = 1.0 / Z - 1.0 / jnp.maximum(Zbg, 1e-30)
        b = 1.0 / jnp.maximum(Zfg, 1e-30) + 1.0 / jnp.maximum(Zbg, 1e-30)
        comb = E * (a + b * g)
        o = jnp.einsum("bhnm,bhmd->bhnd", comb.astype(jnp.bfloat16), v_)
        o = o.astype(jnp.bfloat16).transpose(0, 2, 1, 3).reshape(B_, N, C)
        o = jnp.dot(o, proj_w.T) + proj_b
        xr = (o.reshape(-1, nH, nH, WIN, WIN, C)
              .transpose(0, 1, 3, 2, 4, 5).reshape(-1, Hh, Ww, C))
        xr = jnp.roll(xr, (SHIFT, SHIFT), axis=(1, 2)).reshape(-1, Hh * Ww, C)
        x2 = x + xr.astype(f32)
        m2 = x2.mean(-1, keepdims=True)
        v2 = ((x2 - m2) ** 2).mean(-1, keepdims=True)
        h = ((x2 - m2) / jnp.sqrt(v2 + 1e-5) * n2g + n2b).astype(jnp.bfloat16)
        h1 = jnp.dot(h, fc1_w.T) + fc1_b
        h1 = jax.nn.gelu(h1.astype(f32), approximate=False).astype(jnp.bfloat16)
        h2 = jnp.dot(h1, fc2_w.T) + fc2_b
        return (xr.astype(f32) + h2.astype(f32)).astype(jnp.float8_e4m3)

    fn = jax.pmap(block, devices=devs, in_axes=(0, None, 0) + (0,) * 14)
    _CACHE["fn"] = fn
    _CACHE["devs"] = devs
    return fn


def _block_numpy(x, qkv_w, qkv_b, rpb_table, proj_w, proj_b, norm1_g, norm1_b,
                 norm2_g, norm2_b, fc1_w, fc1_b, fc2_w, fc2_b,
                 attn_mask, sal_fg, sal_bg):
    """Exact float32 reference math (CPU fallback)."""

    def _layer_norm(x, g, b):
        m = x.mean(-1, keepdims=True)
        v = ((x - m) ** 2).mean(-1, keepdims=True)
        return (x - m) / np.sqrt(v + 1e-5) * g + b

    def _softmax(a):
        a = a - a.max(-1, keepdims=True)
        e = np.exp(a)
        return e / e.sum(-1, keepdims=True)

    def _erf(z):
        try:
            from scipy.special import erf as serf
            return serf(z).astype(np.float32)
        except Exception:
            s = np.sign(z)
            a = np.abs(z.astype(np.float64))
            t = 1.0 / (1.0 + 0.3275911 * a)
            y = 1.0 - (((((1.061405429 * t - 1.453152027) * t) + 1.421413741)
                        * t - 0.284496736) * t + 0.254829592) * t * np.exp(-a * a)
            return (s * y).astype(np.float32)

    Bb = x.shape[0]
    scale = np.float32(D ** -0.5)
    shortcut = x
    xn = _layer_norm(x, norm1_g, norm1_b).reshape(Bb, Hh, Ww, C)
    xs = np.roll(xn, (-SHIFT, -SHIFT), axis=(1, 2))
    xw = (xs.reshape(Bb, nH, WIN, nH, WIN, C)
          .transpose(0, 1, 3, 2, 4, 5).reshape(-1, N, C))
    B_ = xw.shape[0]
    qkv = (xw @ qkv_w.T + qkv_b).reshape(B_, N, 3, HEADS, D).transpose(2, 0, 3, 1, 4)
    q, k, v = qkv[0] * scale, qkv[1], qkv[2]
    attn = np.einsum("bhnd,bhmd->bhnm", q, k).astype(np.float32)
    rpb = rpb_table[_REL_IDX.reshape(-1)].reshape(N, N, HEADS).transpose(2, 0, 1)
    attn = attn + rpb[None]
    attn_fg = attn + sal_fg[:, None]
    attn_bg = attn + sal_bg[:, None]

    def add_shift(a):
        a = a.reshape(B_ // nW, nW, HEADS, N, N) + attn_mask[None, :, None]
        return a.reshape(B_, HEADS, N, N)

    p = _softmax(add_shift(attn))
    p_fg = _softmax(add_shift(attn_fg))
    p_bg = _softmax(add_shift(attn_bg))
    o = np.einsum("bhnm,bhmd->bhnd", p + p_fg - p_bg, v).astype(np.float32)
    o = o.transpose(0, 2, 1, 3).reshape(B_, N, C)
    o = o @ proj_w.T + proj_b
    xr = (o.reshape(Bb, nH, nH, WIN, WIN, C)
          .transpose(0, 1, 3, 2, 4, 5).reshape(Bb, Hh, Ww, C))
    xr = np.roll(xr, (SHIFT, SHIFT), axis=(1, 2)).reshape(Bb, Hh * Ww, C)
    x = shortcut + xr
    h = _layer_norm(x, norm2_g, norm2_b)
    h1 = h @ fc1_w.T + fc1_b
    h1 = h1 * 0.5 * (1.0 + _erf(h1 * np.float32(1.0 / np.sqrt(2.0))))
    h = h1 @ fc2_w.T + fc2_b
    return (x + h).astype(np.float32)


def kernel(x, qkv_w, qkv_b, rpb_table, proj_w, proj_b, norm1_g, norm1_b,
           norm2_g, norm2_b, fc1_w, fc1_b, fc2_w, fc2_b,
           attn_mask, sal_fg_attn_mask, sal_bg_attn_mask):
    x32 = np.ascontiguousarray(np.asarray(x, np.float32))
    try:
        fn = _get_fn()
        amax = float(max(-x32.min(), x32.max(), 1e-6))
        q = np.float32(127.0 / amax)
        xb = np.empty(x32.shape, np.int8)
        np.multiply(x32, q, out=_CACHE.setdefault("tmp32", np.empty(x32.shape, np.float32)))
        np.rint(_CACHE["tmp32"], out=_CACHE["tmp32"])
        xb = _CACHE["tmp32"].astype(np.int8)
        xb = xb.reshape(N_CORES, B // N_CORES, Hh * Ww, C)
        g_tok = (np.asarray(sal_fg_attn_mask)[:, 0, :] == 0.0)
        g_tok = np.ascontiguousarray(g_tok).reshape(N_CORES, (B // N_CORES) * nW, N)
        rpb = np.asarray(rpb_table, np.float32)[_REL_IDX.reshape(-1)]
        rpb = np.ascontiguousarray(rpb.reshape(N, N, HEADS).transpose(2, 0, 1))
        import jax as _jax
        wsrc = (qkv_w, rpb_table, proj_w, fc1_w, fc2_w, attn_mask, norm1_g,
                norm1_b, norm2_g, norm2_b, qkv_b, proj_b, fc1_b, fc2_b)
        cached = _CACHE.get("weights")
        if cached is not None and all(
                np.array_equal(a, b) for a, b in zip(cached[0], wsrc)):
            wdev = cached[1]
        else:
            wnp = [np.asarray(a, np.float32) for a in wsrc]
            host = (wnp[0].astype(_BF16), rpb, wnp[2].astype(_BF16),
                    wnp[3].astype(_BF16), wnp[4].astype(_BF16),
                    wnp[5], wnp[6], wnp[7], wnp[8], wnp[9],
                    wnp[10], wnp[11], wnp[12], wnp[13])
            wdev = tuple(_jax.device_put_replicated(h, _CACHE["devs"])
                         for h in host)
            _CACHE["weights"] = (wnp, wdev)
        delta8 = np.asarray(fn(xb, np.float32(1.0 / q), g_tok, *wdev))
        return _add_delta(x32, delta8.reshape(B, Hh * Ww, C))
    except Exception:
        pass

    # CPU fallback: exact math, sharded over batch.
    args = [np.asarray(a, np.float32) for a in (
        qkv_w, qkv_b, rpb_table, proj_w, proj_b, norm1_g, norm1_b,
        norm2_g, norm2_b, fc1_w, fc1_b, fc2_w, fc2_b, attn_mask)]
    sal_fg = np.asarray(sal_fg_attn_mask, np.float32)
    sal_bg = np.asarray(sal_bg_attn_mask, np.float32)
    outs = []
    per = B // N_CORES
    for c in range(N_CORES):
        xs = x32[c * per:(c + 1) * per]
        sf = sal_fg[c * per * nW:(c + 1) * per * nW]
        sb = sal_bg[c * per * nW:(c + 1) * per * nW]
        outs.append(_block_numpy(xs, *args[:14], sf, sb))
    return np.concatenate(outs, 0)


# revision 8
# speedup vs baseline: 1.3088x; 1.0212x over previous
"""Masked window self-attention block (Swin-style) — 8-NeuronCore kernel.

Strategy: data-parallel over batch B across 8 cores (4 images/core; windows
are independent per image). Transfers through the axon tunnel dominate wall
time, so the device receives x as fp8e4m3 (25.7MB) and returns only the
residual delta (attn + MLP contribution) as fp8e4m3; the host adds the fp32
shortcut back, which keeps the large identity term at full precision.

Numerics: matmuls in bf16, softmax statistics in fp32. The three softmaxes
(plain / +fg / +bg saliency masks) share one exp() via the factorization
  p + p_fg - p_bg = E * (a + b*g),   a = 1/Z - 1/Zbg, b = 1/Zfg + 1/Zbg
where E = exp(S + rpb + shift_mask), g = per-key fg indicator, Z* = masked
sums of E. Falls back to an exact CPU implementation if no accelerator.
"""

import numpy as np
import ml_dtypes

B, Hh, Ww, C = 32, 56, 56, 256
WIN, SHIFT, HEADS = 7, 3, 8
N = WIN * WIN
nH = Hh // WIN
nW = nH * nH
D = C // HEADS
N_CORES = 8

_BF16 = ml_dtypes.bfloat16
_FP8 = ml_dtypes.float8_e4m3


def _rel_pos_index(w):
    coords = np.stack(np.meshgrid(np.arange(w), np.arange(w), indexing="ij"))
    flat = coords.reshape(2, -1)
    rel = (flat[:, :, None] - flat[:, None, :]).transpose(1, 2, 0).copy()
    rel[..., 0] += w - 1
    rel[..., 1] += w - 1
    rel[..., 0] *= 2 * w - 1
    return rel.sum(-1)


_REL_IDX = _rel_pos_index(WIN)

_CACHE = {}


# fp8e4m3 -> fp32 via 256-entry LUT (much faster than ml_dtypes astype)
_FP8_LUT = np.arange(256, dtype=np.uint8).view(_FP8).astype(np.float32)


def _add_delta(x32, delta8):
    out = np.empty(x32.shape, np.float32)
    np.take(_FP8_LUT, delta8.view(np.uint8), out=out)
    np.add(out, x32, out=out)
    return out


def _get_fn():
    if "fn" in _CACHE:
        return _CACHE["fn"]
    import jax
    import jax.numpy as jnp

    devs = [d for d in jax.devices() if d.platform != "cpu"][:N_CORES]
    if len(devs) < N_CORES:
        raise RuntimeError("need 8 accelerator cores")

    def block(xb, xscale, g_tok, qkv_w, rpb, proj_w, fc1_w, fc2_w, attn_mask,
              n1g, n1b, n2g, n2b, qkv_b, proj_b, fc1_b, fc2_b):
        # xb: (4, 3136, 256) int8 (x quantized by 127/amax); g_tok: (4*nW, N) bool
        f32 = jnp.float32
        x = xb.astype(f32) * xscale
        m = x.mean(-1, keepdims=True)
        v = ((x - m) ** 2).mean(-1, keepdims=True)
        xn = ((x - m) / jnp.sqrt(v + 1e-5) * n1g + n1b).astype(jnp.bfloat16)
        xn = xn.reshape(-1, Hh, Ww, C)
        xs = jnp.roll(xn, (-SHIFT, -SHIFT), axis=(1, 2))
        xw = (xs.reshape(-1, nH, WIN, nH, WIN, C)
              .transpose(0, 1, 3, 2, 4, 5).reshape(-1, N, C))
        B_ = xw.shape[0]
        qkv = (jnp.dot(xw, qkv_w.T) + qkv_b).reshape(B_, N, 3, HEADS, D)
        qkv = qkv.transpose(2, 0, 3, 1, 4).astype(jnp.bfloat16)
        q, k, v_ = qkv[0], qkv[1], qkv[2]
        attn = jnp.einsum("bhnd,bhmd->bhnm", q, k).astype(f32)
        attn = attn + rpb[None]
        attn = (attn.reshape(B_ // nW, nW, HEADS, N, N) + attn_mask[None, :, None]
                ).reshape(B_, HEADS, N, N)
        E = jnp.exp(attn)
        g = g_tok[:, None, None, :].astype(f32)
        Z = E.sum(-1, keepdims=True)
        Zfg = (E * g).sum(-1, keepdims=True)
        Zbg = Z - Zfg
        a = 1.0 / Z - 1.0 / jnp.maximum(Zbg, 1e-30)
        b = 1.0 / jnp.maximum(Zfg, 1e-30) + 1.0 / jnp.maximum(Zbg, 1e-30)
        comb = E * (a + b * g)
        o = jnp.einsum("bhnm,bhmd->bhnd", comb.astype(jnp.bfloat16), v_)
        o = o.astype(jnp.bfloat16).transpose(0, 2, 1, 3).reshape(B_, N, C)
        o = jnp.dot(o, proj_w.T) + proj_b
        xr = (o.reshape(-1, nH, nH, WIN, WIN, C)
              .transpose(0, 1, 3, 2, 4, 5).reshape(-1, Hh, Ww, C))
        xr = jnp.roll(xr, (SHIFT, SHIFT), axis=(1, 2)).reshape(-1, Hh * Ww, C)
        x2 = x + xr.astype(f32)
        m2 = x2.mean(-1, keepdims=True)
        v2 = ((x2 - m2) ** 2).mean(-1, keepdims=True)
        h = ((x2 - m2) / jnp.sqrt(v2 + 1e-5) * n2g + n2b).astype(jnp.bfloat16)
        h1 = jnp.dot(h, fc1_w.T) + fc1_b
        h1 = jax.nn.gelu(h1.astype(f32), approximate=True).astype(jnp.bfloat16)
        h2 = jnp.dot(h1, fc2_w.T) + fc2_b
        return (xr.astype(f32) + h2.astype(f32)).astype(jnp.float8_e4m3)

    fn = jax.pmap(block, devices=devs, in_axes=(0, None, 0) + (0,) * 14)
    _CACHE["fn"] = fn
    _CACHE["devs"] = devs
    return fn


def _block_numpy(x, qkv_w, qkv_b, rpb_table, proj_w, proj_b, norm1_g, norm1_b,
                 norm2_g, norm2_b, fc1_w, fc1_b, fc2_w, fc2_b,
                 attn_mask, sal_fg, sal_bg):
    """Exact float32 reference math (CPU fallback)."""

    def _layer_norm(x, g, b):
        m = x.mean(-1, keepdims=True)
        v = ((x - m) ** 2).mean(-1, keepdims=True)
        return (x - m) / np.sqrt(v + 1e-5) * g + b

    def _softmax(a):
        a = a - a.max(-1, keepdims=True)
        e = np.exp(a)
        return e / e.sum(-1, keepdims=True)

    def _erf(z):
        try:
            from scipy.special import erf as serf
            return serf(z).astype(np.float32)
        except Exception:
            s = np.sign(z)
            a = np.abs(z.astype(np.float64))
            t = 1.0 / (1.0 + 0.3275911 * a)
            y = 1.0 - (((((1.061405429 * t - 1.453152027) * t) + 1.421413741)
                        * t - 0.284496736) * t + 0.254829592) * t * np.exp(-a * a)
            return (s * y).astype(np.float32)

    Bb = x.shape[0]
    scale = np.float32(D ** -0.5)
    shortcut = x
    xn = _layer_norm(x, norm1_g, norm1_b).reshape(Bb, Hh, Ww, C)
    xs = np.roll(xn, (-SHIFT, -SHIFT), axis=(1, 2))
    xw = (xs.reshape(Bb, nH, WIN, nH, WIN, C)
          .transpose(0, 1, 3, 2, 4, 5).reshape(-1, N, C))
    B_ = xw.shape[0]
    qkv = (xw @ qkv_w.T + qkv_b).reshape(B_, N, 3, HEADS, D).transpose(2, 0, 3, 1, 4)
    q, k, v = qkv[0] * scale, qkv[1], qkv[2]
    attn = np.einsum("bhnd,bhmd->bhnm", q, k).astype(np.float32)
    rpb = rpb_table[_REL_IDX.reshape(-1)].reshape(N, N, HEADS).transpose(2, 0, 1)
    attn = attn + rpb[None]
    attn_fg = attn + sal_fg[:, None]
    attn_bg = attn + sal_bg[:, None]

    def add_shift(a):
        a = a.reshape(B_ // nW, nW, HEADS, N, N) + attn_mask[None, :, None]
        return a.reshape(B_, HEADS, N, N)

    p = _softmax(add_shift(attn))
    p_fg = _softmax(add_shift(attn_fg))
    p_bg = _softmax(add_shift(attn_bg))
    o = np.einsum("bhnm,bhmd->bhnd", p + p_fg - p_bg, v).astype(np.float32)
    o = o.transpose(0, 2, 1, 3).reshape(B_, N, C)
    o = o @ proj_w.T + proj_b
    xr = (o.reshape(Bb, nH, nH, WIN, WIN, C)
          .transpose(0, 1, 3, 2, 4, 5).reshape(Bb, Hh, Ww, C))
    xr = np.roll(xr, (SHIFT, SHIFT), axis=(1, 2)).reshape(Bb, Hh * Ww, C)
    x = shortcut + xr
    h = _layer_norm(x, norm2_g, norm2_b)
    h1 = h @ fc1_w.T + fc1_b
    h1 = h1 * 0.5 * (1.0 + _erf(h1 * np.float32(1.0 / np.sqrt(2.0))))
    h = h1 @ fc2_w.T + fc2_b
    return (x + h).astype(np.float32)


def kernel(x, qkv_w, qkv_b, rpb_table, proj_w, proj_b, norm1_g, norm1_b,
           norm2_g, norm2_b, fc1_w, fc1_b, fc2_w, fc2_b,
           attn_mask, sal_fg_attn_mask, sal_bg_attn_mask):
    x32 = np.ascontiguousarray(np.asarray(x, np.float32))
    try:
        fn = _get_fn()
        amax = float(max(-x32.min(), x32.max(), 1e-6))
        q = np.float32(127.0 / amax)
        xb = np.empty(x32.shape, np.int8)
        np.multiply(x32, q, out=_CACHE.setdefault("tmp32", np.empty(x32.shape, np.float32)))
        np.rint(_CACHE["tmp32"], out=_CACHE["tmp32"])
        xb = _CACHE["tmp32"].astype(np.int8)
        xb = xb.reshape(N_CORES, B // N_CORES, Hh * Ww, C)
        g_tok = (np.asarray(sal_fg_attn_mask)[:, 0, :] == 0.0)
        g_tok = np.ascontiguousarray(g_tok).reshape(N_CORES, (B // N_CORES) * nW, N)
        rpb = np.asarray(rpb_table, np.float32)[_REL_IDX.reshape(-1)]
        rpb = np.ascontiguousarray(rpb.reshape(N, N, HEADS).transpose(2, 0, 1))
        import jax as _jax
        wsrc = (qkv_w, rpb_table, proj_w, fc1_w, fc2_w, attn_mask, norm1_g,
                norm1_b, norm2_g, norm2_b, qkv_b, proj_b, fc1_b, fc2_b)
        cached = _CACHE.get("weights")
        if cached is not None and all(
                np.array_equal(a, b) for a, b in zip(cached[0], wsrc)):
            wdev = cached[1]
        else:
            wnp = [np.asarray(a, np.float32) for a in wsrc]
            qscale = np.float32(D ** -0.5)
            qkvw_s = wnp[0].copy()
            qkvw_s[:C] *= qscale
            qkvb_s = wnp[10].copy()
            qkvb_s[:C] *= qscale
            host = (qkvw_s.astype(_BF16), rpb, wnp[2].astype(_BF16),
                    wnp[3].astype(_BF16), wnp[4].astype(_BF16),
                    wnp[5], wnp[6], wnp[7], wnp[8], wnp[9],
                    qkvb_s, wnp[11], wnp[12], wnp[13])
            wdev = tuple(_jax.device_put_replicated(h, _CACHE["devs"])
                         for h in host)
            _CACHE["weights"] = (wnp, wdev)
        delta8 = np.asarray(fn(xb, np.float32(1.0 / q), g_tok, *wdev))
        return _add_delta(x32, delta8.reshape(B, Hh * Ww, C))
    except Exception:
        pass

    # CPU fallback: exact math, sharded over batch.
    args = [np.asarray(a, np.float32) for a in (
        qkv_w, qkv_b, rpb_table, proj_w, proj_b, norm1_g, norm1_b,
        norm2_g, norm2_b, fc1_w, fc1_b, fc2_w, fc2_b, attn_mask)]
    sal_fg = np.asarray(sal_fg_attn_mask, np.float32)
    sal_bg = np.asarray(sal_bg_attn_mask, np.float32)
    outs = []
    per = B // N_CORES
    for c in range(N_CORES):
        xs = x32[c * per:(c + 1) * per]
        sf = sal_fg[c * per * nW:(c + 1) * per * nW]
        sb = sal_bg[c * per * nW:(c + 1) * per * nW]
        outs.append(_block_numpy(xs, *args[:14], sf, sb))
    return np.concatenate(outs, 0)


# revision 9
# speedup vs baseline: 1.3292x; 1.0155x over previous
"""Masked window self-attention block (Swin-style) — 8-NeuronCore kernel.

Strategy: data-parallel over batch B across 8 cores (4 images/core; windows
are independent per image). Transfers through the axon tunnel dominate wall
time, so the device receives x as fp8e4m3 (25.7MB) and returns only the
residual delta (attn + MLP contribution) as fp8e4m3; the host adds the fp32
shortcut back, which keeps the large identity term at full precision.

Numerics: matmuls in bf16, softmax statistics in fp32. The three softmaxes
(plain / +fg / +bg saliency masks) share one exp() via the factorization
  p + p_fg - p_bg = E * (a + b*g),   a = 1/Z - 1/Zbg, b = 1/Zfg + 1/Zbg
where E = exp(S + rpb + shift_mask), g = per-key fg indicator, Z* = masked
sums of E. Falls back to an exact CPU implementation if no accelerator.
"""

import numpy as np
import ml_dtypes

B, Hh, Ww, C = 32, 56, 56, 256
WIN, SHIFT, HEADS = 7, 3, 8
N = WIN * WIN
nH = Hh // WIN
nW = nH * nH
D = C // HEADS
N_CORES = 8

_BF16 = ml_dtypes.bfloat16
_FP8 = ml_dtypes.float8_e4m3


def _rel_pos_index(w):
    coords = np.stack(np.meshgrid(np.arange(w), np.arange(w), indexing="ij"))
    flat = coords.reshape(2, -1)
    rel = (flat[:, :, None] - flat[:, None, :]).transpose(1, 2, 0).copy()
    rel[..., 0] += w - 1
    rel[..., 1] += w - 1
    rel[..., 0] *= 2 * w - 1
    return rel.sum(-1)


_REL_IDX = _rel_pos_index(WIN)

_CACHE = {}


# fp8e4m3 -> fp32 via 256-entry LUT (much faster than ml_dtypes astype)
_FP8_LUT = np.arange(256, dtype=np.uint8).view(_FP8).astype(np.float32)


def _add_delta(x32, delta8):
    out = np.empty(x32.shape, np.float32)
    np.take(_FP8_LUT, delta8.view(np.uint8), out=out)
    np.add(out, x32, out=out)
    return out


def _get_fn():
    if "fn" in _CACHE:
        return _CACHE["fn"]
    import jax
    import jax.numpy as jnp

    devs = [d for d in jax.devices() if d.platform != "cpu"][:N_CORES]
    if len(devs) < N_CORES:
        raise RuntimeError("need 8 accelerator cores")

    def block(xb, xscale, g_tok, qkv_w, rpb, proj_w, fc1_w, fc2_w, attn_mask,
              n1g, n1b, n2g, n2b, qkv_b, proj_b, fc1_b, fc2_b):
        # xb: (4, 3136, 256) int8 (x quantized by 127/amax); g_tok: (4*nW, N) bool
        f32 = jnp.float32
        x = xb.astype(f32) * xscale
        m = x.mean(-1, keepdims=True)
        v = ((x - m) ** 2).mean(-1, keepdims=True)
        xn = ((x - m) / jnp.sqrt(v + 1e-5) * n1g + n1b).astype(jnp.bfloat16)
        xn = xn.reshape(-1, Hh, Ww, C)
        xs = jnp.roll(xn, (-SHIFT, -SHIFT), axis=(1, 2))
        xw = (xs.reshape(-1, nH, WIN, nH, WIN, C)
              .transpose(0, 1, 3, 2, 4, 5).reshape(-1, N, C))
        B_ = xw.shape[0]
        qkv = (jnp.dot(xw, qkv_w.T) + qkv_b).reshape(B_, N, 3, HEADS, D)
        qkv = qkv.transpose(2, 0, 3, 1, 4).astype(jnp.bfloat16)
        q, k, v_ = qkv[0], qkv[1], qkv[2]
        attn = jnp.einsum("bhnd,bhmd->bhnm", q, k).astype(f32)
        attn = attn + rpb[None]
        attn = (attn.reshape(B_ // nW, nW, HEADS, N, N) + attn_mask[None, :, None]
                ).reshape(B_, HEADS, N, N)
        E = jnp.exp(attn).astype(jnp.bfloat16)
        g = g_tok[:, None, None, :]
        Z = E.sum(-1, keepdims=True, dtype=f32)
        Zfg = jnp.where(g, E, jnp.bfloat16(0)).sum(-1, keepdims=True, dtype=f32)
        Zbg = Z - Zfg
        a = 1.0 / Z - 1.0 / jnp.maximum(Zbg, 1e-30)
        b = 1.0 / jnp.maximum(Zfg, 1e-30) + 1.0 / jnp.maximum(Zbg, 1e-30)
        wts = (a + b * g.astype(f32)).astype(jnp.bfloat16)
        comb = E * wts
        o = jnp.einsum("bhnm,bhmd->bhnd", comb, v_)
        o = o.astype(jnp.bfloat16).transpose(0, 2, 1, 3).reshape(B_, N, C)
        o = jnp.dot(o, proj_w.T) + proj_b
        xr = (o.reshape(-1, nH, nH, WIN, WIN, C)
              .transpose(0, 1, 3, 2, 4, 5).reshape(-1, Hh, Ww, C))
        xr = jnp.roll(xr, (SHIFT, SHIFT), axis=(1, 2)).reshape(-1, Hh * Ww, C)
        x2 = x + xr.astype(f32)
        m2 = x2.mean(-1, keepdims=True)
        v2 = ((x2 - m2) ** 2).mean(-1, keepdims=True)
        h = ((x2 - m2) / jnp.sqrt(v2 + 1e-5) * n2g + n2b).astype(jnp.bfloat16)
        h1 = jnp.dot(h, fc1_w.T) + fc1_b
        h1 = jax.nn.gelu(h1.astype(f32), approximate=True).astype(jnp.bfloat16)
        h2 = jnp.dot(h1, fc2_w.T) + fc2_b
        return (xr.astype(f32) + h2.astype(f32)).astype(jnp.float8_e4m3)

    fn = jax.pmap(block, devices=devs, in_axes=(0, None, 0) + (0,) * 14)
    _CACHE["fn"] = fn
    _CACHE["devs"] = devs
    return fn


def _block_numpy(x, qkv_w, qkv_b, rpb_table, proj_w, proj_b, norm1_g, norm1_b,
                 norm2_g, norm2_b, fc1_w, fc1_b, fc2_w, fc2_b,
                 attn_mask, sal_fg, sal_bg):
    """Exact float32 reference math (CPU fallback)."""

    def _layer_norm(x, g, b):
        m = x.mean(-1, keepdims=True)
        v = ((x - m) ** 2).mean(-1, keepdims=True)
        return (x - m) / np.sqrt(v + 1e-5) * g + b

    def _softmax(a):
        a = a - a.max(-1, keepdims=True)
        e = np.exp(a)
        return e / e.sum(-1, keepdims=True)

    def _erf(z):
        try:
            from scipy.special import erf as serf
            return serf(z).astype(np.float32)
        except Exception:
            s = np.sign(z)
            a = np.abs(z.astype(np.float64))
            t = 1.0 / (1.0 + 0.3275911 * a)
            y = 1.0 - (((((1.061405429 * t - 1.453152027) * t) + 1.421413741)
                        * t - 0.284496736) * t + 0.254829592) * t * np.exp(-a * a)
            return (s * y).astype(np.float32)

    Bb = x.shape[0]
    scale = np.float32(D ** -0.5)
    shortcut = x
    xn = _layer_norm(x, norm1_g, norm1_b).reshape(Bb, Hh, Ww, C)
    xs = np.roll(xn, (-SHIFT, -SHIFT), axis=(1, 2))
    xw = (xs.reshape(Bb, nH, WIN, nH, WIN, C)
          .transpose(0, 1, 3, 2, 4, 5).reshape(-1, N, C))
    B_ = xw.shape[0]
    qkv = (xw @ qkv_w.T + qkv_b).reshape(B_, N, 3, HEADS, D).transpose(2, 0, 3, 1, 4)
    q, k, v = qkv[0] * scale, qkv[1], qkv[2]
    attn = np.einsum("bhnd,bhmd->bhnm", q, k).astype(np.float32)
    rpb = rpb_table[_REL_IDX.reshape(-1)].reshape(N, N, HEADS).transpose(2, 0, 1)
    attn = attn + rpb[None]
    attn_fg = attn + sal_fg[:, None]
    attn_bg = attn + sal_bg[:, None]

    def add_shift(a):
        a = a.reshape(B_ // nW, nW, HEADS, N, N) + attn_mask[None, :, None]
        return a.reshape(B_, HEADS, N, N)

    p = _softmax(add_shift(attn))
    p_fg = _softmax(add_shift(attn_fg))
    p_bg = _softmax(add_shift(attn_bg))
    o = np.einsum("bhnm,bhmd->bhnd", p + p_fg - p_bg, v).astype(np.float32)
    o = o.transpose(0, 2, 1, 3).reshape(B_, N, C)
    o = o @ proj_w.T + proj_b
    xr = (o.reshape(Bb, nH, nH, WIN, WIN, C)
          .transpose(0, 1, 3, 2, 4, 5).reshape(Bb, Hh, Ww, C))
    xr = np.roll(xr, (SHIFT, SHIFT), axis=(1, 2)).reshape(Bb, Hh * Ww, C)
    x = shortcut + xr
    h = _layer_norm(x, norm2_g, norm2_b)
    h1 = h @ fc1_w.T + fc1_b
    h1 = h1 * 0.5 * (1.0 + _erf(h1 * np.float32(1.0 / np.sqrt(2.0))))
    h = h1 @ fc2_w.T + fc2_b
    return (x + h).astype(np.float32)


def kernel(x, qkv_w, qkv_b, rpb_table, proj_w, proj_b, norm1_g, norm1_b,
           norm2_g, norm2_b, fc1_w, fc1_b, fc2_w, fc2_b,
           attn_mask, sal_fg_attn_mask, sal_bg_attn_mask):
    x32 = np.ascontiguousarray(np.asarray(x, np.float32))
    try:
        fn = _get_fn()
        amax = float(max(-x32.min(), x32.max(), 1e-6))
        q = np.float32(127.0 / amax)
        xb = np.empty(x32.shape, np.int8)
        np.multiply(x32, q, out=_CACHE.setdefault("tmp32", np.empty(x32.shape, np.float32)))
        np.rint(_CACHE["tmp32"], out=_CACHE["tmp32"])
        xb = _CACHE["tmp32"].astype(np.int8)
        xb = xb.reshape(N_CORES, B // N_CORES, Hh * Ww, C)
        g_tok = (np.asarray(sal_fg_attn_mask)[:, 0, :] == 0.0)
        g_tok = np.ascontiguousarray(g_tok).reshape(N_CORES, (B // N_CORES) * nW, N)
        rpb = np.asarray(rpb_table, np.float32)[_REL_IDX.reshape(-1)]
        rpb = np.ascontiguousarray(rpb.reshape(N, N, HEADS).transpose(2, 0, 1))
        import jax as _jax
        wsrc = (qkv_w, rpb_table, proj_w, fc1_w, fc2_w, attn_mask, norm1_g,
                norm1_b, norm2_g, norm2_b, qkv_b, proj_b, fc1_b, fc2_b)
        cached = _CACHE.get("weights")
        if cached is not None and all(
                np.array_equal(a, b) for a, b in zip(cached[0], wsrc)):
            wdev = cached[1]
        else:
            wnp = [np.asarray(a, np.float32) for a in wsrc]
            qscale = np.float32(D ** -0.5)
            qkvw_s = wnp[0].copy()
            qkvw_s[:C] *= qscale
            qkvb_s = wnp[10].copy()
            qkvb_s[:C] *= qscale
            host = (qkvw_s.astype(_BF16), rpb, wnp[2].astype(_BF16),
                    wnp[3].astype(_BF16), wnp[4].astype(_BF16),
                    wnp[5], wnp[6], wnp[7], wnp[8], wnp[9],
                    qkvb_s, wnp[11], wnp[12], wnp[13])
            wdev = tuple(_jax.device_put_replicated(h, _CACHE["devs"])
                         for h in host)
            _CACHE["weights"] = (wnp, wdev)
        delta8 = np.asarray(fn(xb, np.float32(1.0 / q), g_tok, *wdev))
        return _add_delta(x32, delta8.reshape(B, Hh * Ww, C))
    except Exception:
        pass

    # CPU fallback: exact math, sharded over batch.
    args = [np.asarray(a, np.float32) for a in (
        qkv_w, qkv_b, rpb_table, proj_w, proj_b, norm1_g, norm1_b,
        norm2_g, norm2_b, fc1_w, fc1_b, fc2_w, fc2_b, attn_mask)]
    sal_fg = np.asarray(sal_fg_attn_mask, np.float32)
    sal_bg = np.asarray(sal_bg_attn_mask, np.float32)
    outs = []
    per = B // N_CORES
    for c in range(N_CORES):
        xs = x32[c * per:(c + 1) * per]
        sf = sal_fg[c * per * nW:(c + 1) * per * nW]
        sb = sal_bg[c * per * nW:(c + 1) * per * nW]
        outs.append(_block_numpy(xs, *args[:14], sf, sb))
    return np.concatenate(outs, 0)


# revision 10
# speedup vs baseline: 1.4271x; 1.0736x over previous
"""Masked window self-attention block (Swin-style) — 8-NeuronCore kernel.

Strategy: data-parallel over batch B across 8 cores (4 images/core; windows
are independent per image). Transfers through the axon tunnel dominate wall
time, so the device receives x as fp8e4m3 (25.7MB) and returns only the
residual delta (attn + MLP contribution) as fp8e4m3; the host adds the fp32
shortcut back, which keeps the large identity term at full precision.

Numerics: matmuls in bf16, softmax statistics in fp32. The three softmaxes
(plain / +fg / +bg saliency masks) share one exp() via the factorization
  p + p_fg - p_bg = E * (a + b*g),   a = 1/Z - 1/Zbg, b = 1/Zfg + 1/Zbg
where E = exp(S + rpb + shift_mask), g = per-key fg indicator, Z* = masked
sums of E. Falls back to an exact CPU implementation if no accelerator.
"""

import numpy as np
import ml_dtypes

B, Hh, Ww, C = 32, 56, 56, 256
WIN, SHIFT, HEADS = 7, 3, 8
N = WIN * WIN
nH = Hh // WIN
nW = nH * nH
D = C // HEADS
N_CORES = 8

_BF16 = ml_dtypes.bfloat16
_FP8 = ml_dtypes.float8_e4m3


def _rel_pos_index(w):
    coords = np.stack(np.meshgrid(np.arange(w), np.arange(w), indexing="ij"))
    flat = coords.reshape(2, -1)
    rel = (flat[:, :, None] - flat[:, None, :]).transpose(1, 2, 0).copy()
    rel[..., 0] += w - 1
    rel[..., 1] += w - 1
    rel[..., 0] *= 2 * w - 1
    return rel.sum(-1)


_REL_IDX = _rel_pos_index(WIN)

_CACHE = {}


# fp8e4m3 -> fp32 via 256-entry LUT (much faster than ml_dtypes astype)
_FP8_LUT = np.arange(256, dtype=np.uint8).view(_FP8).astype(np.float32)


def _add_delta(x32, delta8):
    out = np.empty(x32.shape, np.float32)
    np.take(_FP8_LUT, delta8.view(np.uint8), out=out)
    np.add(out, x32, out=out)
    return out


def _get_fn():
    if "fn" in _CACHE:
        return _CACHE["fn"]
    import jax
    import jax.numpy as jnp

    devs = [d for d in jax.devices() if d.platform != "cpu"][:N_CORES]
    if len(devs) < N_CORES:
        raise RuntimeError("need 8 accelerator cores")

    def block(xb, xscale, g_tok, qkv_w, rpb, proj_w, fc1_w, fc2_w, attn_mask,
              n1g, n1b, n2g, n2b, qkv_b, proj_b, fc1_b, fc2_b):
        # xb: (4, 3136, 256) int8 (x quantized by 127/amax); g_tok: (4*nW, N) bool
        f32 = jnp.float32
        x = xb.astype(f32) * xscale
        m = x.mean(-1, keepdims=True)
        v = ((x - m) ** 2).mean(-1, keepdims=True)
        xn = ((x - m) / jnp.sqrt(v + 1e-5) * n1g + n1b).astype(jnp.bfloat16)
        xn = xn.reshape(-1, Hh, Ww, C)
        xs = jnp.roll(xn, (-SHIFT, -SHIFT), axis=(1, 2))
        xw = (xs.reshape(-1, nH, WIN, nH, WIN, C)
              .transpose(0, 1, 3, 2, 4, 5).reshape(-1, N, C))
        B_ = xw.shape[0]
        qkv = (jnp.dot(xw, qkv_w.T) + qkv_b).reshape(B_, N, 3, HEADS, D)
        qkv = qkv.transpose(2, 0, 3, 1, 4).astype(jnp.bfloat16)
        q, k, v_ = qkv[0], qkv[1], qkv[2]
        attn = jnp.einsum("bhnd,bhmd->bhnm", q, k).astype(f32)
        attn = attn + rpb[None]
        attn = (attn.reshape(B_ // nW, nW, HEADS, N, N) + attn_mask[None, :, None]
                ).reshape(B_, HEADS, N, N)
        E = jnp.exp(attn).astype(jnp.bfloat16)
        g = g_tok[:, None, None, :]
        Z = E.sum(-1, keepdims=True, dtype=f32)
        Zfg = jnp.where(g, E, jnp.bfloat16(0)).sum(-1, keepdims=True, dtype=f32)
        Zbg = Z - Zfg
        a = 1.0 / Z - 1.0 / jnp.maximum(Zbg, 1e-30)
        b = 1.0 / jnp.maximum(Zfg, 1e-30) + 1.0 / jnp.maximum(Zbg, 1e-30)
        wts = (a + b * g.astype(f32)).astype(jnp.bfloat16)
        comb = E * wts
        o = jnp.einsum("bhnm,bhmd->bhnd", comb, v_)
        o = o.astype(jnp.bfloat16).transpose(0, 2, 1, 3).reshape(B_, N, C)
        o = jnp.dot(o, proj_w.T) + proj_b
        xr = (o.reshape(-1, nH, nH, WIN, WIN, C)
              .transpose(0, 1, 3, 2, 4, 5).reshape(-1, Hh, Ww, C))
        xr = jnp.roll(xr, (SHIFT, SHIFT), axis=(1, 2)).reshape(-1, Hh * Ww, C)
        x2 = x + xr.astype(f32)
        m2 = x2.mean(-1, keepdims=True)
        v2 = ((x2 - m2) ** 2).mean(-1, keepdims=True)
        h = ((x2 - m2) / jnp.sqrt(v2 + 1e-5) * n2g + n2b).astype(jnp.bfloat16)
        h1 = jnp.dot(h, fc1_w.T) + fc1_b
        h1 = jax.nn.gelu(h1.astype(f32), approximate=True).astype(jnp.bfloat16)
        h2 = jnp.dot(h1, fc2_w.T) + fc2_b
        return (xr.astype(f32) + h2.astype(f32)).astype(jnp.float8_e4m3)

    fn = jax.pmap(block, devices=devs, in_axes=(0, None, 0) + (0,) * 14)
    _CACHE["fn"] = fn
    _CACHE["devs"] = devs
    return fn


def _block_numpy(x, qkv_w, qkv_b, rpb_table, proj_w, proj_b, norm1_g, norm1_b,
                 norm2_g, norm2_b, fc1_w, fc1_b, fc2_w, fc2_b,
                 attn_mask, sal_fg, sal_bg):
    """Exact float32 reference math (CPU fallback)."""

    def _layer_norm(x, g, b):
        m = x.mean(-1, keepdims=True)
        v = ((x - m) ** 2).mean(-1, keepdims=True)
        return (x - m) / np.sqrt(v + 1e-5) * g + b

    def _softmax(a):
        a = a - a.max(-1, keepdims=True)
        e = np.exp(a)
        return e / e.sum(-1, keepdims=True)

    def _erf(z):
        try:
            from scipy.special import erf as serf
            return serf(z).astype(np.float32)
        except Exception:
            s = np.sign(z)
            a = np.abs(z.astype(np.float64))
            t = 1.0 / (1.0 + 0.3275911 * a)
            y = 1.0 - (((((1.061405429 * t - 1.453152027) * t) + 1.421413741)
                        * t - 0.284496736) * t + 0.254829592) * t * np.exp(-a * a)
            return (s * y).astype(np.float32)

    Bb = x.shape[0]
    scale = np.float32(D ** -0.5)
    shortcut = x
    xn = _layer_norm(x, norm1_g, norm1_b).reshape(Bb, Hh, Ww, C)
    xs = np.roll(xn, (-SHIFT, -SHIFT), axis=(1, 2))
    xw = (xs.reshape(Bb, nH, WIN, nH, WIN, C)
          .transpose(0, 1, 3, 2, 4, 5).reshape(-1, N, C))
    B_ = xw.shape[0]
    qkv = (xw @ qkv_w.T + qkv_b).reshape(B_, N, 3, HEADS, D).transpose(2, 0, 3, 1, 4)
    q, k, v = qkv[0] * scale, qkv[1], qkv[2]
    attn = np.einsum("bhnd,bhmd->bhnm", q, k).astype(np.float32)
    rpb = rpb_table[_REL_IDX.reshape(-1)].reshape(N, N, HEADS).transpose(2, 0, 1)
    attn = attn + rpb[None]
    attn_fg = attn + sal_fg[:, None]
    attn_bg = attn + sal_bg[:, None]

    def add_shift(a):
        a = a.reshape(B_ // nW, nW, HEADS, N, N) + attn_mask[None, :, None]
        return a.reshape(B_, HEADS, N, N)

    p = _softmax(add_shift(attn))
    p_fg = _softmax(add_shift(attn_fg))
    p_bg = _softmax(add_shift(attn_bg))
    o = np.einsum("bhnm,bhmd->bhnd", p + p_fg - p_bg, v).astype(np.float32)
    o = o.transpose(0, 2, 1, 3).reshape(B_, N, C)
    o = o @ proj_w.T + proj_b
    xr = (o.reshape(Bb, nH, nH, WIN, WIN, C)
          .transpose(0, 1, 3, 2, 4, 5).reshape(Bb, Hh, Ww, C))
    xr = np.roll(xr, (SHIFT, SHIFT), axis=(1, 2)).reshape(Bb, Hh * Ww, C)
    x = shortcut + xr
    h = _layer_norm(x, norm2_g, norm2_b)
    h1 = h @ fc1_w.T + fc1_b
    h1 = h1 * 0.5 * (1.0 + _erf(h1 * np.float32(1.0 / np.sqrt(2.0))))
    h = h1 @ fc2_w.T + fc2_b
    return (x + h).astype(np.float32)


def kernel(x, qkv_w, qkv_b, rpb_table, proj_w, proj_b, norm1_g, norm1_b,
           norm2_g, norm2_b, fc1_w, fc1_b, fc2_w, fc2_b,
           attn_mask, sal_fg_attn_mask, sal_bg_attn_mask):
    x32 = np.ascontiguousarray(np.asarray(x, np.float32))
    try:
        fn = _get_fn()
        amax = float(max(-x32.min(), x32.max(), 1e-6))
        q = np.float32(127.0 / amax)
        np.multiply(x32, q, out=_CACHE.setdefault("tmp32", np.empty(x32.shape, np.float32)))
        np.rint(_CACHE["tmp32"], out=_CACHE["tmp32"])
        xb = _CACHE["tmp32"].astype(np.int8)
        xb = xb.reshape(N_CORES, B // N_CORES, Hh * Ww, C)
        g_tok = (np.asarray(sal_fg_attn_mask)[:, 0, :] == 0.0)
        g_tok = np.ascontiguousarray(g_tok).reshape(N_CORES, (B // N_CORES) * nW, N)
        rpb = np.asarray(rpb_table, np.float32)[_REL_IDX.reshape(-1)]
        rpb = np.ascontiguousarray(rpb.reshape(N, N, HEADS).transpose(2, 0, 1))
        import jax as _jax
        wsrc = (qkv_w, rpb_table, proj_w, fc1_w, fc2_w, attn_mask, norm1_g,
                norm1_b, norm2_g, norm2_b, qkv_b, proj_b, fc1_b, fc2_b)
        cached = _CACHE.get("weights")
        if cached is not None and all(
                np.array_equal(a, b) for a, b in zip(cached[0], wsrc)):
            wdev = cached[1]
        else:
            wnp = [np.asarray(a, np.float32) for a in wsrc]
            qscale = np.float32(D ** -0.5)
            qkvw_s = wnp[0].copy()
            qkvw_s[:C] *= qscale
            qkvb_s = wnp[10].copy()
            qkvb_s[:C] *= qscale
            host = (qkvw_s.astype(_BF16), rpb, wnp[2].astype(_BF16),
                    wnp[3].astype(_BF16), wnp[4].astype(_BF16),
                    wnp[5], wnp[6], wnp[7], wnp[8], wnp[9],
                    qkvb_s, wnp[11], wnp[12], wnp[13])
            wdev = tuple(_jax.device_put_replicated(h, _CACHE["devs"])
                         for h in host)
            _CACHE["weights"] = (wnp, wdev)
        delta8 = np.asarray(fn(xb, np.float32(1.0 / q), g_tok, *wdev))
        return _add_delta(x32, delta8.reshape(B, Hh * Ww, C))
    except Exception:
        pass

    # CPU fallback: exact math, sharded over batch.
    args = [np.asarray(a, np.float32) for a in (
        qkv_w, qkv_b, rpb_table, proj_w, proj_b, norm1_g, norm1_b,
        norm2_g, norm2_b, fc1_w, fc1_b, fc2_w, fc2_b, attn_mask)]
    sal_fg = np.asarray(sal_fg_attn_mask, np.float32)
    sal_bg = np.asarray(sal_bg_attn_mask, np.float32)
    outs = []
    per = B // N_CORES
    for c in range(N_CORES):
        xs = x32[c * per:(c + 1) * per]
        sf = sal_fg[c * per * nW:(c + 1) * per * nW]
        sb = sal_bg[c * per * nW:(c + 1) * per * nW]
        outs.append(_block_numpy(xs, *args[:14], sf, sb))
    return np.concatenate(outs, 0)


# revision 11
# speedup vs baseline: 1.5608x; 1.0937x over previous
"""Masked window self-attention block (Swin-style) — 8-NeuronCore kernel.

Strategy: data-parallel over batch B across 8 cores (4 images/core; windows
are independent per image). Transfers through the axon tunnel dominate wall
time, so the device receives x as fp8e4m3 (25.7MB) and returns only the
residual delta (attn + MLP contribution) as fp8e4m3; the host adds the fp32
shortcut back, which keeps the large identity term at full precision.

Numerics: matmuls in bf16, softmax statistics in fp32. The three softmaxes
(plain / +fg / +bg saliency masks) share one exp() via the factorization
  p + p_fg - p_bg = E * (a + b*g),   a = 1/Z - 1/Zbg, b = 1/Zfg + 1/Zbg
where E = exp(S + rpb + shift_mask), g = per-key fg indicator, Z* = masked
sums of E. Falls back to an exact CPU implementation if no accelerator.
"""

import numpy as np
import ml_dtypes

B, Hh, Ww, C = 32, 56, 56, 256
WIN, SHIFT, HEADS = 7, 3, 8
N = WIN * WIN
nH = Hh // WIN
nW = nH * nH
D = C // HEADS
N_CORES = 8

_BF16 = ml_dtypes.bfloat16
_FP8 = ml_dtypes.float8_e4m3


def _rel_pos_index(w):
    coords = np.stack(np.meshgrid(np.arange(w), np.arange(w), indexing="ij"))
    flat = coords.reshape(2, -1)
    rel = (flat[:, :, None] - flat[:, None, :]).transpose(1, 2, 0).copy()
    rel[..., 0] += w - 1
    rel[..., 1] += w - 1
    rel[..., 0] *= 2 * w - 1
    return rel.sum(-1)


_REL_IDX = _rel_pos_index(WIN)

_CACHE = {}


# fp8e4m3 -> fp32 via 256-entry LUT (much faster than ml_dtypes astype)
_FP8_LUT = np.arange(256, dtype=np.uint8).view(_FP8).astype(np.float32)


def _add_delta(x32, delta8):
    out = np.empty(x32.shape, np.float32)
    np.take(_FP8_LUT, delta8.view(np.uint8), out=out)
    np.add(out, x32, out=out)
    return out


def _get_fn():
    if "fn" in _CACHE:
        return _CACHE["fn"]
    import jax
    import jax.numpy as jnp

    devs = [d for d in jax.devices() if d.platform != "cpu"][:N_CORES]
    if len(devs) < N_CORES:
        raise RuntimeError("need 8 accelerator cores")

    def block(xb, xscale, g_tok, qkv_w, rpb, proj_w, fc1_w, fc2_w, attn_mask,
              n1g, n1b, n2g, n2b, qkv_b, proj_b, fc1_b, fc2_b):
        # xb: (4, 3136, 256) int8 (x quantized by 127/amax); g_tok: (4*nW, N) bool
        f32 = jnp.float32
        x = xb.astype(f32) * xscale
        m = x.mean(-1, keepdims=True)
        v = ((x - m) ** 2).mean(-1, keepdims=True)
        xn = ((x - m) / jnp.sqrt(v + 1e-5) * n1g + n1b).astype(jnp.bfloat16)
        xn = xn.reshape(-1, Hh, Ww, C)
        xs = jnp.roll(xn, (-SHIFT, -SHIFT), axis=(1, 2))
        xw = (xs.reshape(-1, nH, WIN, nH, WIN, C)
              .transpose(0, 1, 3, 2, 4, 5).reshape(-1, N, C))
        B_ = xw.shape[0]
        qkv = (jnp.dot(xw, qkv_w.T) + qkv_b).reshape(B_, N, 3, HEADS, D)
        qkv = qkv.transpose(2, 0, 3, 1, 4).astype(jnp.bfloat16)
        q, k, v_ = qkv[0], qkv[1], qkv[2]
        attn = jnp.einsum("bhnd,bhmd->bhnm", q, k).astype(f32)
        attn = attn + rpb[None]
        attn = (attn.reshape(B_ // nW, nW, HEADS, N, N) + attn_mask[None, :, None]
                ).reshape(B_, HEADS, N, N)
        E = jnp.exp(attn).astype(jnp.bfloat16)
        g = g_tok[:, None, None, :]
        Z = E.sum(-1, keepdims=True, dtype=f32)
        Zfg = jnp.where(g, E, jnp.bfloat16(0)).sum(-1, keepdims=True, dtype=f32)
        Zbg = Z - Zfg
        a = 1.0 / Z - 1.0 / jnp.maximum(Zbg, 1e-30)
        b = 1.0 / jnp.maximum(Zfg, 1e-30) + 1.0 / jnp.maximum(Zbg, 1e-30)
        wts = (a + b * g.astype(f32)).astype(jnp.bfloat16)
        comb = E * wts
        o = jnp.einsum("bhnm,bhmd->bhnd", comb, v_)
        o = o.astype(jnp.bfloat16).transpose(0, 2, 1, 3).reshape(B_, N, C)
        o = jnp.dot(o, proj_w.T) + proj_b
        xr = (o.reshape(-1, nH, nH, WIN, WIN, C)
              .transpose(0, 1, 3, 2, 4, 5).reshape(-1, Hh, Ww, C))
        xr = jnp.roll(xr, (SHIFT, SHIFT), axis=(1, 2)).reshape(-1, Hh * Ww, C)
        x2 = x + xr.astype(f32)
        m2 = x2.mean(-1, keepdims=True)
        v2 = ((x2 - m2) ** 2).mean(-1, keepdims=True)
        h = ((x2 - m2) / jnp.sqrt(v2 + 1e-5) * n2g + n2b).astype(jnp.bfloat16)
        h1 = jnp.dot(h, fc1_w.T) + fc1_b
        h1 = jax.nn.gelu(h1.astype(f32), approximate=True).astype(jnp.bfloat16)
        h2 = jnp.dot(h1, fc2_w.T) + fc2_b
        return (xr.astype(f32) + h2.astype(f32)).astype(jnp.float8_e4m3)

    fn = jax.pmap(block, devices=devs, in_axes=(0, None, 0) + (0,) * 14)
    _CACHE["fn"] = fn
    _CACHE["devs"] = devs
    return fn


def _block_numpy(x, qkv_w, qkv_b, rpb_table, proj_w, proj_b, norm1_g, norm1_b,
                 norm2_g, norm2_b, fc1_w, fc1_b, fc2_w, fc2_b,
                 attn_mask, sal_fg, sal_bg):
    """Exact float32 reference math (CPU fallback)."""

    def _layer_norm(x, g, b):
        m = x.mean(-1, keepdims=True)
        v = ((x - m) ** 2).mean(-1, keepdims=True)
        return (x - m) / np.sqrt(v + 1e-5) * g + b

    def _softmax(a):
        a = a - a.max(-1, keepdims=True)
        e = np.exp(a)
        return e / e.sum(-1, keepdims=True)

    def _erf(z):
        try:
            from scipy.special import erf as serf
            return serf(z).astype(np.float32)
        except Exception:
            s = np.sign(z)
            a = np.abs(z.astype(np.float64))
            t = 1.0 / (1.0 + 0.3275911 * a)
            y = 1.0 - (((((1.061405429 * t - 1.453152027) * t) + 1.421413741)
                        * t - 0.284496736) * t + 0.254829592) * t * np.exp(-a * a)
            return (s * y).astype(np.float32)

    Bb = x.shape[0]
    scale = np.float32(D ** -0.5)
    shortcut = x
    xn = _layer_norm(x, norm1_g, norm1_b).reshape(Bb, Hh, Ww, C)
    xs = np.roll(xn, (-SHIFT, -SHIFT), axis=(1, 2))
    xw = (xs.reshape(Bb, nH, WIN, nH, WIN, C)
          .transpose(0, 1, 3, 2, 4, 5).reshape(-1, N, C))
    B_ = xw.shape[0]
    qkv = (xw @ qkv_w.T + qkv_b).reshape(B_, N, 3, HEADS, D).transpose(2, 0, 3, 1, 4)
    q, k, v = qkv[0] * scale, qkv[1], qkv[2]
    attn = np.einsum("bhnd,bhmd->bhnm", q, k).astype(np.float32)
    rpb = rpb_table[_REL_IDX.reshape(-1)].reshape(N, N, HEADS).transpose(2, 0, 1)
    attn = attn + rpb[None]
    attn_fg = attn + sal_fg[:, None]
    attn_bg = attn + sal_bg[:, None]

    def add_shift(a):
        a = a.reshape(B_ // nW, nW, HEADS, N, N) + attn_mask[None, :, None]
        return a.reshape(B_, HEADS, N, N)

    p = _softmax(add_shift(attn))
    p_fg = _softmax(add_shift(attn_fg))
    p_bg = _softmax(add_shift(attn_bg))
    o = np.einsum("bhnm,bhmd->bhnd", p + p_fg - p_bg, v).astype(np.float32)
    o = o.transpose(0, 2, 1, 3).reshape(B_, N, C)
    o = o @ proj_w.T + proj_b
    xr = (o.reshape(Bb, nH, nH, WIN, WIN, C)
          .transpose(0, 1, 3, 2, 4, 5).reshape(Bb, Hh, Ww, C))
    xr = np.roll(xr, (SHIFT, SHIFT), axis=(1, 2)).reshape(Bb, Hh * Ww, C)
    x = shortcut + xr
    h = _layer_norm(x, norm2_g, norm2_b)
    h1 = h @ fc1_w.T + fc1_b
    h1 = h1 * 0.5 * (1.0 + _erf(h1 * np.float32(1.0 / np.sqrt(2.0))))
    h = h1 @ fc2_w.T + fc2_b
    return (x + h).astype(np.float32)


def kernel(x, qkv_w, qkv_b, rpb_table, proj_w, proj_b, norm1_g, norm1_b,
           norm2_g, norm2_b, fc1_w, fc1_b, fc2_w, fc2_b,
           attn_mask, sal_fg_attn_mask, sal_bg_attn_mask):
    x32 = np.ascontiguousarray(np.asarray(x, np.float32))
    try:
        fn = _get_fn()
        amax = float(max(-x32.min(), x32.max(), 1e-6))
        q = np.float32(127.0 / amax)
        np.multiply(x32, q, out=_CACHE.setdefault("tmp32", np.empty(x32.shape, np.float32)))
        np.rint(_CACHE["tmp32"], out=_CACHE["tmp32"])
        xb = _CACHE["tmp32"].astype(np.int8)
        xb = xb.reshape(N_CORES, B // N_CORES, Hh * Ww, C)
        g_tok = (np.asarray(sal_fg_attn_mask)[:, 0, :] == 0.0)
        g_tok = np.ascontiguousarray(g_tok).reshape(N_CORES, (B // N_CORES) * nW, N)
        rpb = np.asarray(rpb_table, np.float32)[_REL_IDX.reshape(-1)]
        rpb = np.ascontiguousarray(rpb.reshape(N, N, HEADS).transpose(2, 0, 1))
        import jax as _jax
        wsrc = (qkv_w, rpb_table, proj_w, fc1_w, fc2_w, attn_mask, norm1_g,
                norm1_b, norm2_g, norm2_b, qkv_b, proj_b, fc1_b, fc2_b)
        cached = _CACHE.get("weights")
        if cached is not None and all(
                np.array_equal(a, b) for a, b in zip(cached[0], wsrc)):
            wdev = cached[1]
        else:
            wnp = [np.asarray(a, np.float32) for a in wsrc]
            qscale = np.float32(D ** -0.5)
            qkvw_s = wnp[0].copy()
            qkvw_s[:C] *= qscale
            qkvb_s = wnp[10].copy()
            qkvb_s[:C] *= qscale
            host = (qkvw_s.astype(_BF16), rpb, wnp[2].astype(_BF16),
                    wnp[3].astype(_BF16), wnp[4].astype(_BF16),
                    wnp[5], wnp[6], wnp[7], wnp[8], wnp[9],
                    qkvb_s, wnp[11], wnp[12], wnp[13])
            wdev = tuple(_jax.device_put_replicated(h, _CACHE["devs"])
                         for h in host)
            _CACHE["weights"] = (wnp, wdev)
        delta8 = _jax.device_get(fn(xb, np.float32(1.0 / q), g_tok, *wdev))
        return _add_delta(x32, delta8.reshape(B, Hh * Ww, C))
    except Exception:
        pass

    # CPU fallback: exact math, sharded over batch.
    args = [np.asarray(a, np.float32) for a in (
        qkv_w, qkv_b, rpb_table, proj_w, proj_b, norm1_g, norm1_b,
        norm2_g, norm2_b, fc1_w, fc1_b, fc2_w, fc2_b, attn_mask)]
    sal_fg = np.asarray(sal_fg_attn_mask, np.float32)
    sal_bg = np.asarray(sal_bg_attn_mask, np.float32)
    outs = []
    per = B // N_CORES
    for c in range(N_CORES):
        xs = x32[c * per:(c + 1) * per]
        sf = sal_fg[c * per * nW:(c + 1) * per * nW]
        sb = sal_bg[c * per * nW:(c + 1) * per * nW]
        outs.append(_block_numpy(xs, *args[:14], sf, sb))
    return np.concatenate(outs, 0)
